# revision 6
# baseline (speedup 1.0000x reference)
import sys

for _p in ("/opt/trn_rl_repo", "/root/.axon_site/_ro/trn_rl_repo"):
    if _p not in sys.path:
        sys.path.insert(0, _p)

import numpy as np
import concourse.bass as bass
import concourse.bacc as bacc
import concourse.tile as tile
import concourse.mybir as mybir
from concourse.bass_utils import run_bass_kernel_spmd

F32 = mybir.dt.float32
F32R = mybir.dt.float32r

B, T, C = 8, 4096, 32
L = 25
P = L // 2          # 12
NT = T // 128       # 32 s-tiles / t-tiles
PAD = 128           # zero pad columns on each side of xT
XTW = T + 2 * PAD   # padded xT width
WIN = 152           # per-tile window width: 128 + 2*P
QW = (NT + 2) * 32  # q_sb width incl. one zero tile each side
EPS = 1e-12

_CACHE = {}


def _host_consts(w, proj_w):
    wt = np.zeros(256, dtype=np.float32)  # w~[delta+128] = w[delta+12], |delta|<=12
    for d in range(-P, P + 1):
        wt[d + 128] = w[d + P]

    ai = np.subtract.outer(np.arange(128), np.arange(128))  # a - i
    bandM = wt[np.clip(ai, -128, 127) + 128].astype(np.float32)
    bandL = wt[np.clip(ai - 128, -128, 127) + 128].astype(np.float32)
    bandR = wt[np.clip(ai + 128, -128, 127) + 128].astype(np.float32)

    band1 = wt[np.clip(ai - 12, -128, 127) + 128].astype(np.float32)  # w~[a-12-i]
    a2 = np.subtract.outer(np.arange(32), np.arange(128))
    band2 = wt[np.clip(a2 + 116, -128, 127) + 128].astype(np.float32)  # w~[116+a-i]
    ij = np.subtract.outer(np.arange(128), np.arange(WIN))  # i - j'
    bandT2 = wt[np.clip(-ij - 12, -128, 127) + 128].astype(np.float32)  # w~[j'-12-i]

    # W1[d, n*32+c] = proj_w[c*32+d, n]
    pr3 = proj_w.reshape(C, C, C)  # [c, d, n]
    w1 = np.ascontiguousarray(pr3.transpose(1, 2, 0).reshape(C, C * C)).astype(np.float32)
    return dict(bandM=bandM, bandL=bandL, bandR=bandR, band1=band1,
                band2=band2, bandT2=bandT2, w1=w1)


def _build():
    nc = bacc.Bacc("TRN2", target_bir_lowering=False, debug=False)
    din = lambda n, s, dt=F32: nc.dram_tensor(n, s, dt, kind="ExternalInput")
    xtp_d = din("xtp", [C, XTW])
    xtpr_d = din("xtpr", [C, XTW])
    xblk_d = din("xblk", [128, NT * C])
    w1_d = din("w1", [C, C * C])
    bm_d = din("bandM", [128, 128])
    bl_d = din("bandL", [128, 128])
    br_d = din("bandR", [128, 128])
    b1_d = din("band1", [128, 128])
    b2_d = din("band2", [32, 128])
    bt2_d = din("bandT2", [128, WIN])
    out_d = nc.dram_tensor("out", [T, C], F32, kind="ExternalOutput")

    with tile.TileContext(nc) as tc:
        with tc.tile_pool(name="consts", bufs=1) as cp:
            xtp = cp.tile([C, XTW], F32, tag="xtp")
            xtpr = cp.tile([C, XTW], F32, tag="xtpr")
            xblk = cp.tile([128, NT * C], F32, tag="xblk")
            w1 = cp.tile([C, C * C], F32, tag="w1")
            bm = cp.tile([128, 128], F32, tag="bm")
            bl = cp.tile([128, 128], F32, tag="bl")
            br = cp.tile([128, 128], F32, tag="br")
            b1 = cp.tile([128, 128], F32, tag="b1")
            b2 = cp.tile([32, 128], F32, tag="b2")
            bt2 = cp.tile([128, WIN], F32, tag="bt2")
            for t_, d_ in [(xtp, xtp_d), (xtpr, xtpr_d), (xblk, xblk_d),
                           (w1, w1_d), (bm, bm_d), (bl, bl_d), (br, br_d),
                           (b1, b1_d), (b2, b2_d), (bt2, bt2_d)]:
                nc.sync.dma_start(out=t_, in_=d_[:, :])

            q_sb = cp.tile([128, QW], F32, tag="q")
            nc.vector.memset(q_sb[:, 0:32], 0.0)
            nc.vector.memset(q_sb[:, (NT + 1) * 32:QW], 0.0)
            ns_sb = cp.tile([128, NT], F32, tag="ns")
            eps_t = cp.tile([128, 1], F32, tag="eps")
            nc.vector.memset(eps_t, EPS)

            # ---- phase A: q[s,n] and norm^2 per s-tile ----
            with tc.tile_pool(name="rps", bufs=2, space="PSUM") as rp, \
                 tc.tile_pool(name="gps", bufs=1, space="PSUM") as gp, \
                 tc.tile_pool(name="g2ps", bufs=1, space="PSUM") as g2p, \
                 tc.tile_pool(name="mps", bufs=1, space="PSUM") as mp, \
                 tc.tile_pool(name="sb", bufs=3) as sp, \
                 tc.tile_pool(name="dsb", bufs=2) as dp:
                for k in range(NT):
                    base = PAD + 128 * k
                    # R[s,(n,c)] = sum_d x[s,d] W1[d,(n,c)]  (exact fp32)
                    r_ps = rp.tile([128, 1024], F32, tag="r")
                    nc.tensor.matmul(r_ps[:, 0:512], xtp[:, base:base + 128],
                                     w1[:, 0:512], start=True, stop=True)
                    nc.tensor.matmul(r_ps[:, 512:1024], xtp[:, base:base + 128],
                                     w1[:, 512:1024], start=True, stop=True)
                    # prod = R * x[s,c] (broadcast over n); q = sum_c
                    prod = sp.tile([128, 1024], F32, tag="prod")
                    x_in1 = bass.AP(tensor=xblk.tensor, offset=xblk.offset + k * 32,
                                    ap=[xblk.ap[0], [0, 32], [1, 32]])
                    nc.vector.tensor_tensor(out=prod, in0=r_ps, in1=x_in1,
                                            op=mybir.AluOpType.mult)
                    nc.vector.tensor_reduce(
                        out=q_sb[:, (k + 1) * 32:(k + 2) * 32],
                        in_=prod.rearrange("p (n c) -> p n c", c=32),
                        axis=mybir.AxisListType.X, op=mybir.AluOpType.add)

                    # Gram blocks (fp32r): rows s=t0-12+a, cols s'=t0-12+j'
                    wbase = base - P
                    g1 = gp.tile([128, WIN], F32, tag="g1")
                    nc.tensor.matmul(g1, xtpr[:, wbase:wbase + 128],
                                     xtpr[:, wbase:wbase + WIN], start=True, stop=True)
                    g2 = g2p.tile([32, WIN], F32, tag="g2")
                    nc.tensor.matmul(g2, xtpr[:, base + 116:base + 116 + 32],
                                     xtpr[:, wbase:wbase + WIN], start=True, stop=True)
                    d1 = dp.tile([128, WIN], F32, tag="d1")
                    nc.scalar.activation(out=d1, in_=g1,
                                         func=mybir.ActivationFunctionType.Square)
                    d2 = dp.tile([32, WIN], F32, tag="d2")
                    nc.scalar.activation(out=d2, in_=g2,
                                         func=mybir.ActivationFunctionType.Square)
                    m1 = mp.tile([128, WIN], F32, tag="m1")
                    nc.tensor.matmul(m1, b1, d1, start=True, stop=False)
                    nc.tensor.matmul(m1, b2, d2, start=False, stop=True)
                    scr = sp.tile([128, WIN], F32, tag="scr")
                    nc.vector.tensor_tensor(out=scr, in0=m1, in1=bt2,
                                            op=mybir.AluOpType.mult)
                    nc.vector.tensor_reduce(out=ns_sb[:, k:k + 1], in_=scr,
                                            axis=mybir.AxisListType.X,
                                            op=mybir.AluOpType.add)

            # ---- rsqrt + one Newton step ----
            r0 = cp.tile([128, NT], F32, tag="r0")
            nc.scalar.activation(out=r0, in_=ns_sb,
                                 func=mybir.ActivationFunctionType.Sqrt,
                                 bias=eps_t, scale=1.0)
            nc.vector.reciprocal(out=r0, in_=r0)
            t1 = cp.tile([128, NT], F32, tag="t1")
            nc.vector.tensor_tensor(out=t1, in0=r0, in1=r0, op=mybir.AluOpType.mult)
            nc.vector.tensor_tensor(out=t1, in0=t1, in1=ns_sb, op=mybir.AluOpType.mult)
            nc.vector.tensor_scalar(out=t1, in0=t1, scalar1=-0.5, scalar2=1.5,
                                    op0=mybir.AluOpType.mult, op1=mybir.AluOpType.add)
            rs = cp.tile([128, NT], F32, tag="rs")
            nc.vector.tensor_tensor(out=rs, in0=r0, in1=t1, op=mybir.AluOpType.mult)

            # ---- phase B: g = Band-conv(q); out = g * rsqrt ----
            with tc.tile_pool(name="gcps", bufs=1, space="PSUM") as gc, \
                 tc.tile_pool(name="osb", bufs=3) as op_:
                g_ps = gc.tile([128, NT * 32], F32, tag="g")
                for k in range(NT):
                    for bnd, off, st, sp_ in ((bm, 1, True, False),
                                              (bl, 0, False, False),
                                              (br, 2, False, True)):
                        nc.tensor.matmul(g_ps[:, k * 32:(k + 1) * 32], bnd,
                                         q_sb[:, (k + off) * 32:(k + off + 1) * 32],
                                         start=st, stop=sp_)
                for k in range(NT):
                    ot = op_.tile([128, 32], F32, tag="ot")
                    nc.vector.tensor_scalar(out=ot, in0=g_ps[:, k * 32:(k + 1) * 32],
                                            scalar1=rs[:, k:k + 1], scalar2=None,
                                            op0=mybir.AluOpType.mult)
                    nc.sync.dma_start(out=out_d[128 * k:128 * (k + 1), :], in_=ot)
    nc.compile()
    return nc


def kernel(x, w, proj_w, proj_b):
    x = np.asarray(x, dtype=np.float32)
    w = np.asarray(w, dtype=np.float32)
    proj_w = np.asarray(proj_w, dtype=np.float32)
    proj_b = np.asarray(proj_b, dtype=np.float32)

    consts = _host_consts(w, proj_w)
    if "nc" not in _CACHE:
        _CACHE["nc"] = _build()
    nc = _CACHE["nc"]

    in_maps = []
    for b in range(B):
        xb = x[b]  # [T, C]
        xtp = np.zeros((C, XTW), dtype=np.float32)
        xtp[:, PAD:PAD + T] = xb.T
        xblk = np.ascontiguousarray(
            xb.reshape(NT, 128, C).transpose(1, 0, 2).reshape(128, NT * C))
        m = {"xtp": xtp, "xtpr": xtp, "xblk": xblk}
        m.update({k: np.ascontiguousarray(v) for k, v in consts.items()})
        in_maps.append(m)

    res = run_bass_kernel_spmd(nc, in_maps, list(range(B)))
    out = np.stack([res.results[b]["out"] for b in range(B)], axis=0)
    return (out + proj_b[None, None, :]).astype(np.float32)


if __name__ == "__main__":
    rng = np.random.default_rng(0)
    x = rng.standard_normal((B, T, C), dtype=np.float32)
    w = rng.standard_normal(L).astype(np.float32)
    pw = (rng.standard_normal((C * C, C)) * 0.02).astype(np.float32)
    pb = np.zeros(C, dtype=np.float32)
    o = kernel(x, w, pw, pb)
    print("out", o.shape, o.dtype, np.abs(o).max())


# revision 7
# speedup vs baseline: 1.1607x; 1.1607x over previous
import sys

for _p in ("/opt/trn_rl_repo", "/root/.axon_site/_ro/trn_rl_repo"):
    if _p not in sys.path:
        sys.path.insert(0, _p)

import numpy as np
import concourse.bass as bass
import concourse.bacc as bacc
import concourse.tile as tile
import concourse.mybir as mybir
from concourse.bass_utils import run_bass_kernel_spmd

F32 = mybir.dt.float32
F32R = mybir.dt.float32r

B, T, C = 8, 4096, 32
L = 25
P = L // 2          # 12
NT = T // 128       # 32 s-tiles / t-tiles
PAD = 128           # zero pad columns on each side of xT
XTW = T + 2 * PAD   # padded xT width
WIN = 152           # per-tile window width: 128 + 2*P
QW = (NT + 2) * 32  # q_sb width incl. one zero tile each side
EPS = 1e-12

_CACHE = {}


def _host_consts(w, proj_w):
    wt = np.zeros(256, dtype=np.float32)  # w~[delta+128] = w[delta+12], |delta|<=12
    for d in range(-P, P + 1):
        wt[d + 128] = w[d + P]

    ai = np.subtract.outer(np.arange(128), np.arange(128))  # a - i
    bandM = wt[np.clip(ai, -128, 127) + 128].astype(np.float32)
    bandL = wt[np.clip(ai - 128, -128, 127) + 128].astype(np.float32)
    bandR = wt[np.clip(ai + 128, -128, 127) + 128].astype(np.float32)

    band1 = wt[np.clip(ai - 12, -128, 127) + 128].astype(np.float32)  # w~[a-12-i]
    a2 = np.subtract.outer(np.arange(32), np.arange(128))
    band2 = wt[np.clip(a2 + 116, -128, 127) + 128].astype(np.float32)  # w~[116+a-i]
    ij = np.subtract.outer(np.arange(128), np.arange(WIN))  # i - j'
    bandT2 = wt[np.clip(-ij - 12, -128, 127) + 128].astype(np.float32)  # w~[j'-12-i]

    # W1[d, n*32+c] = proj_w[c*32+d, n]
    pr3 = proj_w.reshape(C, C, C)  # [c, d, n]
    w1 = np.ascontiguousarray(pr3.transpose(1, 2, 0).reshape(C, C * C)).astype(np.float32)
    return dict(bandM=bandM, bandL=bandL, bandR=bandR, band1=band1,
                band2=band2, bandT2=bandT2, w1=w1)


def _build():
    nc = bacc.Bacc("TRN2", target_bir_lowering=False, debug=False)
    din = lambda n, s, dt=F32: nc.dram_tensor(n, s, dt, kind="ExternalInput")
    xtp_d = din("xtp", [C, XTW])
    xblk_d = din("xblk", [128, NT * C])
    w1_d = din("w1", [C, C * C])
    bm_d = din("bandM", [128, 128])
    bl_d = din("bandL", [128, 128])
    br_d = din("bandR", [128, 128])
    b1_d = din("band1", [128, 128])
    b2_d = din("band2", [32, 128])
    bt2_d = din("bandT2", [128, WIN])
    out_d = nc.dram_tensor("out", [T, C], F32, kind="ExternalOutput")

    with tile.TileContext(nc) as tc:
        with tc.tile_pool(name="consts", bufs=1) as cp:
            xtp = cp.tile([C, XTW], F32, tag="xtp")
            xblk = cp.tile([128, NT * C], F32, tag="xblk")
            w1 = cp.tile([C, C * C], F32, tag="w1")
            bm = cp.tile([128, 128], F32, tag="bm")
            bl = cp.tile([128, 128], F32, tag="bl")
            br = cp.tile([128, 128], F32, tag="br")
            b1 = cp.tile([128, 128], F32, tag="b1")
            b2 = cp.tile([32, 128], F32, tag="b2")
            bt2 = cp.tile([128, WIN], F32, tag="bt2")
            for t_, d_ in [(xtp, xtp_d), (xblk, xblk_d),
                           (w1, w1_d), (bm, bm_d), (bl, bl_d), (br, br_d),
                           (b1, b1_d), (b2, b2_d), (bt2, bt2_d)]:
                nc.sync.dma_start(out=t_, in_=d_[:, :])

            q_sb = cp.tile([128, QW], F32, tag="q")
            nc.vector.memset(q_sb[:, 0:32], 0.0)
            nc.vector.memset(q_sb[:, (NT + 1) * 32:QW], 0.0)
            ns_sb = cp.tile([128, NT], F32, tag="ns")
            eps_t = cp.tile([128, 1], F32, tag="eps")
            nc.vector.memset(eps_t, EPS)

            # ---- phase A: q[s,n] and norm^2 per s-tile ----
            with tc.tile_pool(name="rps", bufs=2, space="PSUM") as rp, \
                 tc.tile_pool(name="gps", bufs=1, space="PSUM") as gp, \
                 tc.tile_pool(name="g2ps", bufs=1, space="PSUM") as g2p, \
                 tc.tile_pool(name="mps", bufs=1, space="PSUM") as mp, \
                 tc.tile_pool(name="sb", bufs=3) as sp, \
                 tc.tile_pool(name="dsb", bufs=2) as dp:
                for k in range(NT):
                    base = PAD + 128 * k
                    # R[s,(n,c)] = sum_d x[s,d] W1[d,(n,c)]  (exact fp32)
                    r_ps = rp.tile([128, 1024], F32, tag="r")
                    nc.tensor.matmul(r_ps[:, 0:512], xtp[:, base:base + 128],
                                     w1[:, 0:512], start=True, stop=True)
                    nc.tensor.matmul(r_ps[:, 512:1024], xtp[:, base:base + 128],
                                     w1[:, 512:1024], start=True, stop=True)
                    # prod = R * x[s,c] (broadcast over n); q = sum_c
                    prod = sp.tile([128, 1024], F32, tag="prod")
                    x_in1 = bass.AP(tensor=xblk.tensor, offset=xblk.offset + k * 32,
                                    ap=[xblk.ap[0], [0, 32], [1, 32]])
                    nc.vector.tensor_tensor(out=prod, in0=r_ps, in1=x_in1,
                                            op=mybir.AluOpType.mult)
                    nc.vector.tensor_reduce(
                        out=q_sb[:, (k + 1) * 32:(k + 2) * 32],
                        in_=prod.rearrange("p (n c) -> p n c", c=32),
                        axis=mybir.AxisListType.X, op=mybir.AluOpType.add)

                    # Gram blocks (fp32r): rows s=t0-12+a, cols s'=t0-12+j'
                    wbase = base - P
                    g1 = gp.tile([128, WIN], F32, tag="g1")
                    nc.tensor.matmul(g1, xtp[:, wbase:wbase + 128],
                                     xtp[:, wbase:wbase + WIN], start=True, stop=True)
                    g2 = g2p.tile([32, WIN], F32, tag="g2")
                    nc.tensor.matmul(g2, xtp[:, base + 116:base + 116 + 32],
                                     xtp[:, wbase:wbase + WIN], start=True, stop=True)
                    d1 = dp.tile([128, WIN], F32, tag="d1")
                    nc.scalar.activation(out=d1, in_=g1,
                                         func=mybir.ActivationFunctionType.Square)
                    d2 = dp.tile([32, WIN], F32, tag="d2")
                    nc.scalar.activation(out=d2, in_=g2,
                                         func=mybir.ActivationFunctionType.Square)
                    m1 = mp.tile([128, WIN], F32, tag="m1")
                    nc.tensor.matmul(m1, b1, d1, start=True, stop=False)
                    nc.tensor.matmul(m1, b2, d2, start=False, stop=True)
                    scr = sp.tile([128, WIN], F32, tag="scr")
                    nc.vector.tensor_tensor(out=scr, in0=m1, in1=bt2,
                                            op=mybir.AluOpType.mult)
                    nc.vector.tensor_reduce(out=ns_sb[:, k:k + 1], in_=scr,
                                            axis=mybir.AxisListType.X,
                                            op=mybir.AluOpType.add)

            # ---- rsqrt + one Newton step ----
            r0 = cp.tile([128, NT], F32, tag="r0")
            nc.scalar.activation(out=r0, in_=ns_sb,
                                 func=mybir.ActivationFunctionType.Sqrt,
                                 bias=eps_t, scale=1.0)
            nc.vector.reciprocal(out=r0, in_=r0)
            t1 = cp.tile([128, NT], F32, tag="t1")
            nc.vector.tensor_tensor(out=t1, in0=r0, in1=r0, op=mybir.AluOpType.mult)
            nc.vector.tensor_tensor(out=t1, in0=t1, in1=ns_sb, op=mybir.AluOpType.mult)
            nc.vector.tensor_scalar(out=t1, in0=t1, scalar1=-0.5, scalar2=1.5,
                                    op0=mybir.AluOpType.mult, op1=mybir.AluOpType.add)
            rs = cp.tile([128, NT], F32, tag="rs")
            nc.vector.tensor_tensor(out=rs, in0=r0, in1=t1, op=mybir.AluOpType.mult)

            # ---- phase B: g = Band-conv(q); out = g * rsqrt ----
            with tc.tile_pool(name="gcps", bufs=1, space="PSUM") as gc, \
                 tc.tile_pool(name="osb", bufs=3) as op_:
                g_ps = gc.tile([128, NT * 32], F32, tag="g")
                for k in range(NT):
                    for bnd, off, st, sp_ in ((bm, 1, True, False),
                                              (bl, 0, False, False),
                                              (br, 2, False, True)):
                        nc.tensor.matmul(g_ps[:, k * 32:(k + 1) * 32], bnd,
                                         q_sb[:, (k + off) * 32:(k + off + 1) * 32],
                                         start=st, stop=sp_)
                for k in range(NT):
                    ot = op_.tile([128, 32], F32, tag="ot")
                    nc.vector.tensor_scalar(out=ot, in0=g_ps[:, k * 32:(k + 1) * 32],
                                            scalar1=rs[:, k:k + 1], scalar2=None,
                                            op0=mybir.AluOpType.mult)
                    nc.sync.dma_start(out=out_d[128 * k:128 * (k + 1), :], in_=ot)
    nc.compile()
    return nc


def kernel(x, w, proj_w, proj_b):
    x = np.asarray(x, dtype=np.float32)
    w = np.asarray(w, dtype=np.float32)
    proj_w = np.asarray(proj_w, dtype=np.float32)
    proj_b = np.asarray(proj_b, dtype=np.float32)

    consts = _host_consts(w, proj_w)
    if "nc" not in _CACHE:
        _CACHE["nc"] = _build()
    nc = _CACHE["nc"]

    in_maps = []
    for b in range(B):
        xb = x[b]  # [T, C]
        xtp = np.zeros((C, XTW), dtype=np.float32)
        xtp[:, PAD:PAD + T] = xb.T
        xblk = np.ascontiguousarray(
            xb.reshape(NT, 128, C).transpose(1, 0, 2).reshape(128, NT * C))
        m = {"xtp": xtp, "xblk": xblk}
        m.update({k: np.ascontiguousarray(v) for k, v in consts.items()})
        in_maps.append(m)

    res = run_bass_kernel_spmd(nc, in_maps, list(range(B)))
    out = np.stack([res.results[b]["out"] for b in range(B)], axis=0)
    return (out + proj_b[None, None, :]).astype(np.float32)


if __name__ == "__main__":
    rng = np.random.default_rng(0)
    x = rng.standard_normal((B, T, C), dtype=np.float32)
    w = rng.standard_normal(L).astype(np.float32)
    pw = (rng.standard_normal((C * C, C)) * 0.02).astype(np.float32)
    pb = np.zeros(C, dtype=np.float32)
    o = kernel(x, w, pw, pb)
    print("out", o.shape, o.dtype, np.abs(o).max())


# revision 8
# speedup vs baseline: 1.1679x; 1.0062x over previous
import sys

for _p in ("/opt/trn_rl_repo", "/root/.axon_site/_ro/trn_rl_repo"):
    if _p not in sys.path:
        sys.path.insert(0, _p)

import numpy as np
import concourse.bass as bass
import concourse.bacc as bacc
import concourse.tile as tile
import concourse.mybir as mybir
from concourse.bass_utils import run_bass_kernel_spmd

F32 = mybir.dt.float32
F32R = mybir.dt.float32r

B, T, C = 8, 4096, 32
L = 25
P = L // 2          # 12
NT = T // 128       # 32 s-tiles / t-tiles
PAD = 128           # zero pad columns on each side of xT
XTW = T + 2 * PAD   # padded xT width
WIN = 152           # per-tile window width: 128 + 2*P
QW = (NT + 2) * 32  # q_sb width incl. one zero tile each side
EPS = 1e-12

_CACHE = {}


def _host_consts(w, proj_w):
    wt = np.zeros(256, dtype=np.float32)  # w~[delta+128] = w[delta+12], |delta|<=12
    for d in range(-P, P + 1):
        wt[d + 128] = w[d + P]

    ai = np.subtract.outer(np.arange(128), np.arange(128))  # a - i
    bandM = wt[np.clip(ai, -128, 127) + 128].astype(np.float32)
    bandL = wt[np.clip(ai - 128, -128, 127) + 128].astype(np.float32)
    bandR = wt[np.clip(ai + 128, -128, 127) + 128].astype(np.float32)

    band1 = wt[np.clip(ai - 12, -128, 127) + 128].astype(np.float32)  # w~[a-12-i]
    a2 = np.subtract.outer(np.arange(32), np.arange(128))
    band2 = wt[np.clip(a2 + 116, -128, 127) + 128].astype(np.float32)  # w~[116+a-i]
    ij = np.subtract.outer(np.arange(128), np.arange(WIN))  # i - j'
    bandT2 = wt[np.clip(-ij - 12, -128, 127) + 128].astype(np.float32)  # w~[j'-12-i]

    # W1[d, n*32+c] = proj_w[c*32+d, n]
    pr3 = proj_w.reshape(C, C, C)  # [c, d, n]
    w1 = np.ascontiguousarray(pr3.transpose(1, 2, 0).reshape(C, C * C)).astype(np.float32)
    return dict(bandM=bandM, bandL=bandL, bandR=bandR, band1=band1,
                band2=band2, bandT2=bandT2, w1=w1)


def _build():
    nc = bacc.Bacc("TRN2", target_bir_lowering=False, debug=False)
    din = lambda n, s, dt=F32: nc.dram_tensor(n, s, dt, kind="ExternalInput")
    xtp_d = din("xtp", [C, XTW])
    xblk_d = din("xblk", [128, NT * C])
    w1_d = din("w1", [C, C * C])
    bm_d = din("bandM", [128, 128])
    bl_d = din("bandL", [128, 128])
    br_d = din("bandR", [128, 128])
    b1_d = din("band1", [128, 128])
    b2_d = din("band2", [32, 128])
    bt2_d = din("bandT2", [128, WIN])
    out_d = nc.dram_tensor("out", [T, C], F32, kind="ExternalOutput")

    with tile.TileContext(nc) as tc:
        with tc.tile_pool(name="consts", bufs=1) as cp:
            xtp = cp.tile([C, XTW], F32, tag="xtp")
            xblk = cp.tile([128, NT * C], F32, tag="xblk")
            w1 = cp.tile([C, C * C], F32, tag="w1")
            bm = cp.tile([128, 128], F32, tag="bm")
            bl = cp.tile([128, 128], F32, tag="bl")
            br = cp.tile([128, 128], F32, tag="br")
            b1 = cp.tile([128, 128], F32, tag="b1")
            b2 = cp.tile([32, 128], F32, tag="b2")
            bt2 = cp.tile([128, WIN], F32, tag="bt2")
            for t_, d_ in [(xtp, xtp_d), (xblk, xblk_d),
                           (w1, w1_d), (bm, bm_d), (bl, bl_d), (br, br_d),
                           (b1, b1_d), (b2, b2_d), (bt2, bt2_d)]:
                nc.sync.dma_start(out=t_, in_=d_[:, :])

            q_sb = cp.tile([128, QW], F32, tag="q")
            nc.vector.memset(q_sb[:, 0:32], 0.0)
            nc.vector.memset(q_sb[:, (NT + 1) * 32:QW], 0.0)
            ns_sb = cp.tile([128, NT], F32, tag="ns")
            eps_t = cp.tile([128, 1], F32, tag="eps")
            nc.vector.memset(eps_t, EPS)

            # ---- phase A: q[s,n] and norm^2 per s-tile ----
            with tc.tile_pool(name="rps", bufs=2, space="PSUM") as rp, \
                 tc.tile_pool(name="gps", bufs=1, space="PSUM") as gp, \
                 tc.tile_pool(name="g2ps", bufs=1, space="PSUM") as g2p, \
                 tc.tile_pool(name="mps", bufs=1, space="PSUM") as mp, \
                 tc.tile_pool(name="sb", bufs=3) as sp, \
                 tc.tile_pool(name="dsb", bufs=2) as dp:
                for k in range(NT):
                    base = PAD + 128 * k
                    # R[s,(n,c)] = sum_d x[s,d] W1[d,(n,c)]  (exact fp32)
                    r_ps = rp.tile([128, 1024], F32, tag="r")
                    nc.tensor.matmul(r_ps[:, 0:512], xtp[:, base:base + 128],
                                     w1[:, 0:512], start=True, stop=True)
                    nc.tensor.matmul(r_ps[:, 512:1024], xtp[:, base:base + 128],
                                     w1[:, 512:1024], start=True, stop=True)
                    # prod = R * x[s,c] (broadcast over n); q = sum_c
                    prod = sp.tile([128, 1024], F32, tag="prod")
                    x_in1 = bass.AP(tensor=xblk.tensor, offset=xblk.offset + k * 32,
                                    ap=[xblk.ap[0], [0, 32], [1, 32]])
                    if k % 2 == 0:
                        nc.vector.tensor_tensor(out=prod, in0=r_ps, in1=x_in1,
                                                op=mybir.AluOpType.mult)
                    else:
                        r_sb = sp.tile([128, 1024], F32, tag="rsb")
                        nc.scalar.copy(out=r_sb, in_=r_ps)
                        nc.gpsimd.tensor_tensor(out=prod, in0=r_sb, in1=x_in1,
                                                op=mybir.AluOpType.mult)
                    nc.vector.tensor_reduce(
                        out=q_sb[:, (k + 1) * 32:(k + 2) * 32],
                        in_=prod.rearrange("p (n c) -> p n c", c=32),
                        axis=mybir.AxisListType.X, op=mybir.AluOpType.add)

                    # Gram blocks (fp32r): rows s=t0-12+a, cols s'=t0-12+j'
                    wbase = base - P
                    g1 = gp.tile([128, WIN], F32, tag="g1")
                    nc.tensor.matmul(g1, xtp[:, wbase:wbase + 128],
                                     xtp[:, wbase:wbase + WIN], start=True, stop=True)
                    g2 = g2p.tile([32, WIN], F32, tag="g2")
                    nc.tensor.matmul(g2, xtp[:, base + 116:base + 116 + 32],
                                     xtp[:, wbase:wbase + WIN], start=True, stop=True)
                    d1 = dp.tile([128, WIN], F32, tag="d1")
                    nc.scalar.activation(out=d1, in_=g1,
                                         func=mybir.ActivationFunctionType.Square)
                    d2 = dp.tile([32, WIN], F32, tag="d2")
                    nc.scalar.activation(out=d2, in_=g2,
                                         func=mybir.ActivationFunctionType.Square)
                    m1 = mp.tile([128, WIN], F32, tag="m1")
                    nc.tensor.matmul(m1, b1, d1, start=True, stop=False)
                    nc.tensor.matmul(m1, b2, d2, start=False, stop=True)
                    scr = sp.tile([128, WIN], F32, tag="scr")
                    nc.vector.tensor_tensor(out=scr, in0=m1, in1=bt2,
                                            op=mybir.AluOpType.mult)
                    nc.vector.tensor_reduce(out=ns_sb[:, k:k + 1], in_=scr,
                                            axis=mybir.AxisListType.X,
                                            op=mybir.AluOpType.add)

            # ---- rsqrt + one Newton step ----
            r0 = cp.tile([128, NT], F32, tag="r0")
            nc.scalar.activation(out=r0, in_=ns_sb,
                                 func=mybir.ActivationFunctionType.Sqrt,
                                 bias=eps_t, scale=1.0)
            nc.vector.reciprocal(out=r0, in_=r0)
            t1 = cp.tile([128, NT], F32, tag="t1")
            nc.vector.tensor_tensor(out=t1, in0=r0, in1=r0, op=mybir.AluOpType.mult)
            nc.vector.tensor_tensor(out=t1, in0=t1, in1=ns_sb, op=mybir.AluOpType.mult)
            nc.vector.tensor_scalar(out=t1, in0=t1, scalar1=-0.5, scalar2=1.5,
                                    op0=mybir.AluOpType.mult, op1=mybir.AluOpType.add)
            rs = cp.tile([128, NT], F32, tag="rs")
            nc.vector.tensor_tensor(out=rs, in0=r0, in1=t1, op=mybir.AluOpType.mult)

            # ---- phase B: g = Band-conv(q); out = g * rsqrt ----
            with tc.tile_pool(name="gcps", bufs=1, space="PSUM") as gc, \
                 tc.tile_pool(name="osb", bufs=3) as op_:
                g_ps = gc.tile([128, NT * 32], F32, tag="g")
                for k in range(NT):
                    for bnd, off, st, sp_ in ((bm, 1, True, False),
                                              (bl, 0, False, False),
                                              (br, 2, False, True)):
                        nc.tensor.matmul(g_ps[:, k * 32:(k + 1) * 32], bnd,
                                         q_sb[:, (k + off) * 32:(k + off + 1) * 32],
                                         start=st, stop=sp_)
                for k in range(NT):
                    ot = op_.tile([128, 32], F32, tag="ot")
                    nc.vector.tensor_scalar(out=ot, in0=g_ps[:, k * 32:(k + 1) * 32],
                                            scalar1=rs[:, k:k + 1], scalar2=None,
                                            op0=mybir.AluOpType.mult)
                    nc.sync.dma_start(out=out_d[128 * k:128 * (k + 1), :], in_=ot)
    nc.compile()
    return nc


def kernel(x, w, proj_w, proj_b):
    x = np.asarray(x, dtype=np.float32)
    w = np.asarray(w, dtype=np.float32)
    proj_w = np.asarray(proj_w, dtype=np.float32)
    proj_b = np.asarray(proj_b, dtype=np.float32)

    consts = _host_consts(w, proj_w)
    if "nc" not in _CACHE:
        _CACHE["nc"] = _build()
    nc = _CACHE["nc"]

    in_maps = []
    for b in range(B):
        xb = x[b]  # [T, C]
        xtp = np.zeros((C, XTW), dtype=np.float32)
        xtp[:, PAD:PAD + T] = xb.T
        xblk = np.ascontiguousarray(
            xb.reshape(NT, 128, C).transpose(1, 0, 2).reshape(128, NT * C))
        m = {"xtp": xtp, "xblk": xblk}
        m.update({k: np.ascontiguousarray(v) for k, v in consts.items()})
        in_maps.append(m)

    res = run_bass_kernel_spmd(nc, in_maps, list(range(B)))
    out = np.stack([res.results[b]["out"] for b in range(B)], axis=0)
    return (out + proj_b[None, None, :]).astype(np.float32)


if __name__ == "__main__":
    rng = np.random.default_rng(0)
    x = rng.standard_normal((B, T, C), dtype=np.float32)
    w = rng.standard_normal(L).astype(np.float32)
    pw = (rng.standard_normal((C * C, C)) * 0.02).astype(np.float32)
    pb = np.zeros(C, dtype=np.float32)
    o = kernel(x, w, pw, pb)
    print("out", o.shape, o.dtype, np.abs(o).max())
